# revision 1
# baseline (speedup 1.0000x reference)
"""Trainium2 Bass kernel for nn_MixtureOfMambaModel.

Strategy (8 NeuronCores):
  - Pure data-parallel over batch: 2 batch rows per core, no collectives.
  - Feature-major activation layout on device: [feature_on_partitions, token]
    with token = b*98 + s (196 tokens per core). Weights in natural [in, out]
    layout serve directly as matmul lhsT (stationary); activations stream as
    rhs, so every projection produces the next feature-major activation.
  - Matmuls run in bf16 (f32 PSUM accumulation); SSM scan / norm scalars /
    routing run in f32. Set BASS_MM_DT=f32 for a full-f32 fallback.
  - The Mamba SSM recurrence is a single hardware instruction per batch
    (tensor_tensor_scan: state = a*state + u along the free dim).
  - RMSNorm weights fold into the consuming projection weights on the host
    (pure weight preprocessing); biases are applied via K=1 "ones-row"
    matmul accumulation steps and skipped entirely when the bias is zero.
  - Top-2 softmax routing computed exactly with max/compare/sigmoid ops
    (softmax of 2 values == sigmoid of their difference); dense per-expert
    weights broadcast across partitions with tiny selector matmuls.
  - Expert weights stream from HBM as 0.5MB bf16 slabs, multi-buffered.

Host-side work is limited to data movement / layout (slicing, transposes,
embedding-row gather, dtype casts) and constant folding on weights.
"""

import os
import numpy as np
import ml_dtypes

# ---- model dims (hardcoded per spec) ----
B = 16; TV = 32; CV = 512; HWSP = 196; TA = 64; FA = 128; QL = 20
D = 1024; INNER = 2048; NS = 64; HID = 4096; E = 4; L = 4; NCLS = 13; V = 5000
S = 98                      # sequence = 1 + TV + TA + 1
NCORES = 8
NB = B // NCORES            # batches per core = 2
T = NB * S                  # tokens per core = 196
DC = D // 128               # 8 chunks of D
IC = INNER // 128           # 16 chunks of INNER
HC = HID // 128             # 32 chunks of HID

BF16 = ml_dtypes.bfloat16

_CACHE = {}


def _mm_np():
    return np.float32 if os.environ.get("BASS_MM_DT") == "f32" else BF16


def _fp8_on():
    return os.environ.get("BASS_MM_DT") != "f32" and \
        os.environ.get("BASS_FP8") == "1"


FSCALE = 128.0  # fp8e3m4 weight scale (compensated via ACT scale params)


# --------------------------------------------------------------------------
# Host-side preparation: layout/casts/folding only.
# --------------------------------------------------------------------------

def _prep(inputs):
    mmnp = _mm_np()
    f32 = np.float32
    g = {k: np.asarray(v) for k, v in inputs.items()}

    sh = {}
    flags = {}

    # ---- mixer weights ----
    w_in = g["in_w"] * g["norm1_w"][:, :, None]                      # [L,1024,4096]
    sh["w_in"] = np.ascontiguousarray(
        w_in.reshape(L, DC, 128, 2, 2048).transpose(0, 1, 3, 2, 4)).astype(mmnp)
    flags["b_in"] = bool(np.any(g["in_b"]))
    sh["b_in"] = np.asarray(g["in_b"]).reshape(L, 1, 2 * INNER).astype(mmnp)

    # conv pack: taps w0,w1,w2 + conv bias + D_param -> [L, 128, 16, 5]
    cw = g["conv_w"][:, :, 0, :]                                      # [L,INNER,3]
    cpack = np.zeros((L, 128, IC, 5), f32)
    for t in range(3):
        cpack[:, :, :, t] = cw[:, :, t].reshape(L, IC, 128).transpose(0, 2, 1)
    cpack[:, :, :, 3] = g["conv_b"].reshape(L, IC, 128).transpose(0, 2, 1)
    cpack[:, :, :, 4] = g["D_param"].reshape(L, IC, 128).transpose(0, 2, 1)
    sh["cpack"] = cpack
    flags["b_conv"] = bool(np.any(g["conv_b"]))
    flags["dp_ones"] = bool(np.all(g["D_param"] == 1.0))

    # dt/B/C fused: [L, 2048, 192] -> slabs [L, 4, 128, 4, 192]
    wdtbc = np.concatenate([g["dt_w"], g["Bp_w"], g["Cp_w"]], axis=-1)
    sh["w_dtbc"] = np.ascontiguousarray(
        wdtbc.reshape(L, 4, 4, 128, 192).transpose(0, 1, 3, 2, 4)).astype(mmnp)
    bdtbc = np.concatenate([g["dt_b"], g["Bp_b"], g["Cp_b"]], axis=-1)  # [L,192]
    sh["b_dtbc"] = bdtbc.reshape(L, 1, 192).astype(mmnp)
    flags["b_dtbc"] = bool(np.any(bdtbc))

    # s2i with bias as 65th contraction row: [L, 65, 2048]
    sh["w_s2i"] = np.concatenate(
        [g["s2i_w"], g["s2i_b"][:, None, :]], axis=1).astype(mmnp)

    # out_w: [L, 2048, 1024] -> [L, 2(mh), 8(slab), 128, 2(ktile), 512]
    sh["w_out"] = np.ascontiguousarray(
        g["out_w"].reshape(L, 8, 2, 128, 2, 512)
        .transpose(0, 4, 1, 3, 2, 5)).astype(mmnp)
    sh["b_out"] = g["out_b"].reshape(L, 1, D).astype(mmnp)
    flags["b_out"] = bool(np.any(g["out_b"]))

    # ---- MoE ----
    w_gate = g["gate_w"] * g["norm2_w"][:, :, None]                   # [L,1024,4]
    sh["w_gate"] = np.ascontiguousarray(
        w_gate.reshape(L, DC, 128, E).transpose(0, 2, 1, 3)).astype(mmnp)
    sh["b_gate"] = g["gate_b"].reshape(L, 1, E).astype(mmnp)
    flags["b_gate"] = bool(np.any(g["gate_b"]))

    if _fp8_on():
        e8 = ml_dtypes.float8_e3m4
        wsc = FSCALE
    else:
        e8 = mmnp
        wsc = 1.0
    w_e1 = g["e_w1"] * g["norm2_w"][:, None, :, None] * wsc           # [L,E,1024,4096]
    sh["w_e1"] = np.ascontiguousarray(
        w_e1.reshape(L, E, DC, 128, 2, 2048).transpose(0, 1, 2, 4, 3, 5)).astype(e8)
    sh["b_e1"] = (g["e_b1"] * wsc).reshape(L, E, 1, HID).astype(mmnp)
    flags["b_e1"] = bool(np.any(g["e_b1"]))

    # e_w2: [L, E, 4096, 1024] -> [L, E, 2(mh), 16(slab), 128, 2(ktile), 512]
    sh["w_e2"] = np.ascontiguousarray(
        (g["e_w2"] * wsc).reshape(L, E, 16, 2, 128, 2, 512)
        .transpose(0, 1, 5, 2, 4, 3, 6)).astype(e8)
    sh["b_e2"] = (g["e_b2"] * wsc).reshape(L, E, 1, D).astype(mmnp)
    flags["b_e2"] = bool(np.any(g["e_b2"]))

    # ---- pre/post projections: [K/128, 128, D] slab lists ----
    sh["w_vp"] = np.ascontiguousarray(
        g["video_proj_w"].reshape(4, 128, D)).astype(mmnp)
    sh["b_vp"] = g["video_proj_b"].reshape(1, D).astype(mmnp)
    flags["b_vp"] = bool(np.any(g["video_proj_b"]))
    sh["w_ap"] = np.ascontiguousarray(
        g["audio_proj_w"].reshape(1, 128, D)).astype(mmnp)
    sh["b_ap"] = g["audio_proj_b"].reshape(1, D).astype(mmnp)
    flags["b_ap"] = bool(np.any(g["audio_proj_b"]))
    sh["w_qp"] = np.ascontiguousarray(
        g["q_proj_w"].reshape(DC, 128, D)).astype(mmnp)
    sh["b_qp"] = g["q_proj_b"].reshape(1, D).astype(mmnp)
    flags["b_qp"] = bool(np.any(g["q_proj_b"]))

    w_hd = g["head_w"] * g["fnorm_w"][:, None]                        # [1024, 13]
    sh["w_hd"] = np.ascontiguousarray(
        w_hd.reshape(DC, 128, NCLS).transpose(1, 0, 2)).astype(mmnp)  # [128,8,13]
    sh["b_hd"] = g["head_b"].reshape(1, NCLS).astype(mmnp)
    flags["b_hd"] = bool(np.any(g["head_b"]))

    # additive init constants: cls/modality/pos combined -> [128, 8, 98]
    me = np.asarray(g["modality_embed"][0], f32)                      # [4, 1024]
    pos = np.asarray(g["pos_embed"][0, :S], f32)                      # [98, 1024]
    xadd = np.ascontiguousarray(pos.T).astype(f32)                    # [1024, 98]
    xadd[:, 0] += np.asarray(g["cls_token"][0, 0], f32) + me[3]
    xadd[:, 1:1 + TV] += me[0][:, None]
    xadd[:, 1 + TV:1 + TV + TA] += me[1][:, None]
    xadd[:, 1 + TV + TA] += me[2]
    sh["xadd"] = np.ascontiguousarray(
        xadd.reshape(DC, 128, S).transpose(1, 0, 2))                  # [128, 8, 98]

    # selector constants for router broadcast: [8, 8*128]
    sel = np.zeros((8, 8 * 128), f32)
    for j in range(8):
        sel[j, j * 128:(j + 1) * 128] = 1.0
    sh["sel"] = sel

    # ---- per-core inputs ----
    vid_f = g["video"].reshape(B, TV, CV, HWSP)
    q_rows = g["q_emb"][np.asarray(g["question"])]                    # [B, QL, D]
    percore = []
    for c in range(NCORES):
        cb = c * NB
        pc = {}
        pc["vid"] = np.ascontiguousarray(
            vid_f[cb:cb + NB].reshape(NB, TV, 4, 128, HWSP)
            .transpose(0, 1, 3, 2, 4)).astype(mmnp)                   # [2,32,128,4,196]
        pc["aud"] = np.ascontiguousarray(
            g["audio"][cb:cb + NB].transpose(2, 0, 1).reshape(FA, NB * TA)
        ).astype(mmnp)                                                # [128, 128]
        pc["qg"] = np.ascontiguousarray(
            q_rows[cb:cb + NB].transpose(2, 0, 1)
            .reshape(DC, 128, NB, QL).transpose(1, 0, 2, 3)).astype(f32)
        percore.append(pc)

    return sh, percore, flags


# --------------------------------------------------------------------------
# Device kernel builder
# --------------------------------------------------------------------------

def _build(flags_key):
    import concourse.mybir as mybir
    import concourse.tile as tile
    from concourse import bacc
    from concourse.masks import make_identity

    flags = dict(flags_key)
    MMDT = mybir.dt.float32 if os.environ.get("BASS_MM_DT") == "f32" \
        else mybir.dt.bfloat16
    W8DT = mybir.dt.float8e3 if _fp8_on() else MMDT
    F32 = mybir.dt.float32
    AF = mybir.ActivationFunctionType
    OP = mybir.AluOpType
    AX = mybir.AxisListType

    nc = bacc.Bacc("TRN2", target_bir_lowering=False, debug=False)

    def din(name, shape, dt=MMDT):
        return nc.dram_tensor(name, list(shape), dt, kind="ExternalInput")

    t_w_in = din("w_in", [L, DC, 2, 128, 2048])
    t_b_in = din("b_in", [L, 1, 2 * INNER])
    t_cpack = din("cpack", [L, 128, IC, 5], F32)
    t_w_dtbc = din("w_dtbc", [L, 4, 128, 4, 192])
    t_b_dtbc = din("b_dtbc", [L, 1, 192])
    t_w_s2i = din("w_s2i", [L, NS + 1, INNER])
    t_w_out = din("w_out", [L, 2, 8, 128, 2, 512])
    t_b_out = din("b_out", [L, 1, D])
    t_w_gate = din("w_gate", [L, 128, DC, E])
    t_b_gate = din("b_gate", [L, 1, E])
    t_w_e1 = din("w_e1", [L, E, DC, 2, 128, 2048], W8DT)
    t_b_e1 = din("b_e1", [L, E, 1, HID])
    t_w_e2 = din("w_e2", [L, E, 2, 16, 128, 2, 512], W8DT)
    t_b_e2 = din("b_e2", [L, E, 1, D])
    t_w_vp = din("w_vp", [4, 128, D]); t_b_vp = din("b_vp", [1, D])
    t_w_ap = din("w_ap", [1, 128, D]); t_b_ap = din("b_ap", [1, D])
    t_w_qp = din("w_qp", [DC, 128, D]); t_b_qp = din("b_qp", [1, D])
    t_w_hd = din("w_hd", [128, DC, NCLS]); t_b_hd = din("b_hd", [1, NCLS])
    t_xadd = din("xadd", [128, DC, S], F32)
    t_sel = din("sel", [8, 8 * 128], F32)
    t_vid = din("vid", [NB, TV, 128, 4, HWSP])
    t_aud = din("aud", [128, NB * TA])
    t_qg = din("qg", [128, DC, NB, QL], F32)
    t_out = nc.dram_tensor("out", [NCLS, NB], F32, kind="ExternalOutput")

    taps_on = bool(os.environ.get("BASS_TAPS"))
    tap_handles = {}

    with tile.TileContext(nc) as tc:
        with tc.tile_pool(name="consts", bufs=1) as consts, \
             tc.tile_pool(name="wpre", bufs=8) as wpre, \
             tc.tile_pool(name="wsl", bufs=11) as wsl, \
             tc.tile_pool(name="wsl2", bufs=17) as wsl2, \
             tc.tile_pool(name="wdt", bufs=5) as wdtp, \
             tc.tile_pool(name="wmi", bufs=2) as wmi, \
             tc.tile_pool(name="bsl", bufs=2) as bsl, \
             tc.tile_pool(name="vtp", bufs=4) as vtp, \
             tc.tile_pool(name="p1", bufs=1) as p1, \
             tc.tile_pool(name="p2", bufs=2) as p2, \
             tc.tile_pool(name="psA", bufs=5, space="PSUM") as psA, \
             tc.tile_pool(name="psB", bufs=2, space="PSUM") as psB:

            # ---------- constants ----------
            ones_mm = consts.tile([1, T], MMDT)
            nc.vector.memset(ones_mm[:], 1.0)
            ones128 = consts.tile([128, 128], MMDT)   # rmsnorm reduction lhsT
            nc.vector.memset(ones128[:], 1.0)
            ones64 = consts.tile([NS, 128], F32)      # LN reduction lhsT
            nc.vector.memset(ones64[:], 1.0)
            ident = consts.tile([S, S], F32)
            make_identity(nc, ident[:])
            sel_t = consts.tile([8, 8 * 128], F32)
            nc.gpsimd.dma_start(out=sel_t[:], in_=t_sel.ap())
            xadd_t = consts.tile([128, DC, S], F32)
            nc.gpsimd.dma_start(out=xadd_t[:], in_=t_xadd.ap())
            whd = consts.tile([128, DC, NCLS], MMDT)
            nc.gpsimd.dma_start(out=whd[:], in_=t_w_hd.ap())

            _cregs = {}

            def creg(val):   # [128,1] f32 constant for ACT scale/bias operands
                if val not in _cregs:
                    ct = consts.tile([128, 1], F32, tag=f"c{len(_cregs)}")
                    nc.vector.memset(ct[:], val)
                    _cregs[val] = ct
                return _cregs[val][:]

            def tap(name, ap, dt=None):
                if not taps_on:
                    return
                th = nc.dram_tensor(f"tap_{name}", list(ap.shape),
                                    dt or ap.dtype, kind="ExternalOutput")
                tap_handles[name] = th
                nc.sync.dma_start(out=th.ap(), in_=ap)

            def V4(ap):  # [128, C, 196] -> [128, C, 2, 98]
                return ap.rearrange("p c (b s) -> p c b s", b=NB)

            def bias_row(dram_ap, tag="brow"):
                bt = bsl.tile([1, dram_ap.shape[-1]], MMDT, tag=tag)
                nc.gpsimd.dma_start(out=bt[:], in_=dram_ap)
                return bt

            def load_slabs(dram_ap, n, shape, pool=wsl, tag="wsl", dt=None):
                out = []
                for k in range(n):
                    wt = pool.tile(shape, dt or MMDT, tag=tag)
                    nc.sync.dma_start(out=wt[:], in_=dram_ap[k])
                    out.append(wt)
                return out

            # residual stream x: [128, 8, 196] f32
            x_sb = p1.tile([128, DC, T], F32, tag="x")

            # ---------- preprocessing ----------
            vsum = p1.tile([128, 4, NB * TV], F32, tag="vsum")
            vdm = p1.tile([128, HWSP], F32, tag="vdummy")
            for b in range(NB):
                for tv in range(TV):
                    vt = vtp.tile([128, 4, HWSP], MMDT, tag="vt")
                    nc.sync.dma_start(out=vt[:], in_=t_vid.ap()[b, tv])
                    col = b * TV + tv
                    if tv < 20:   # DVE share of the spatial-mean reduction
                        nc.vector.tensor_reduce(
                            out=vsum[:, :, col:col + 1], in_=vt[:],
                            axis=AX.X, op=OP.add)
                    else:         # ScalarE share via activation accumulate
                        for ct in range(4):
                            nc.scalar.activation(
                                vdm[:], vt[:, ct, :], AF.Copy,
                                accum_out=vsum[:, ct, col:col + 1])
            vsum_m = p1.tile([128, 4, NB * TV], MMDT, tag="vsum_m")
            nc.vector.tensor_scalar(
                out=vsum_m[:], in0=vsum[:], scalar1=1.0 / HWSP, scalar2=None,
                op0=OP.mult)

            NV = NB * TV
            wvp = load_slabs(t_w_vp.ap(), 4, [128, D], wpre, "wpre")
            bvp = bias_row(t_b_vp.ap()) if flags["b_vp"] else None
            for m in range(DC):
                ps = psB.tile([128, T], F32, tag="pmisc")
                for k in range(4):
                    nc.tensor.matmul(
                        ps[:, 0:NV], wvp[k][:, m * 128:(m + 1) * 128],
                        vsum_m[:, k, :], start=(k == 0),
                        stop=(k == 3 and bvp is None))
                if bvp is not None:
                    nc.tensor.matmul(ps[:, 0:NV], bvp[:, m * 128:(m + 1) * 128],
                                     ones_mm[:, 0:NV], start=False, stop=True)
                nc.scalar.copy(x_sb[:, m, 1:1 + TV], ps[:, 0:TV])
                nc.scalar.copy(x_sb[:, m, S + 1:S + 1 + TV], ps[:, TV:2 * TV])

            # audio
            aud_t = p2.tile([128, NB * TA], MMDT, tag="aud")
            nc.gpsimd.dma_start(out=aud_t[:], in_=t_aud.ap())
            wap = load_slabs(t_w_ap.ap(), 1, [128, D], wpre, "wpre")
            bap = bias_row(t_b_ap.ap()) if flags["b_ap"] else None
            NA = NB * TA
            for m in range(DC):
                ps = psB.tile([128, T], F32, tag="pmisc")
                nc.tensor.matmul(ps[:, 0:NA], wap[0][:, m * 128:(m + 1) * 128],
                                 aud_t[:], start=True, stop=(bap is None))
                if bap is not None:
                    nc.tensor.matmul(ps[:, 0:NA], bap[:, m * 128:(m + 1) * 128],
                                     ones_mm[:, 0:NA], start=False, stop=True)
                nc.scalar.copy(x_sb[:, m, 1 + TV:1 + TV + TA], ps[:, 0:TA])
                nc.scalar.copy(x_sb[:, m, S + 1 + TV:S + 1 + TV + TA],
                               ps[:, TA:2 * TA])

            # question: mean of host-gathered embedding rows, then proj
            qg_t = p2.tile([128, DC, NB, QL], F32, tag="qg")
            nc.gpsimd.dma_start(out=qg_t[:], in_=t_qg.ap())
            qred = p2.tile([128, DC, NB], F32, tag="qred")
            nc.vector.tensor_reduce(out=qred[:].unsqueeze(3), in_=qg_t[:],
                                    axis=AX.X, op=OP.add)
            qm = p2.tile([128, DC, NB], MMDT, tag="qm")
            nc.vector.tensor_scalar(out=qm[:], in0=qred[:], scalar1=1.0 / QL,
                                    scalar2=None, op0=OP.mult)
            wqp = load_slabs(t_w_qp.ap(), DC, [128, D], wpre, "wpre")
            bqp = bias_row(t_b_qp.ap()) if flags["b_qp"] else None
            for m in range(DC):
                ps = psB.tile([128, T], F32, tag="pmisc")
                for k in range(DC):
                    nc.tensor.matmul(
                        ps[:, 0:NB], wqp[k][:, m * 128:(m + 1) * 128],
                        qm[:, k, :], start=(k == 0),
                        stop=(k == DC - 1 and bqp is None))
                if bqp is not None:
                    nc.tensor.matmul(ps[:, 0:NB], bqp[:, m * 128:(m + 1) * 128],
                                     ones_mm[:, 0:NB], start=False, stop=True)
                nc.scalar.copy(x_sb[:, m, S - 1:S], ps[:, 0:1])
                nc.scalar.copy(x_sb[:, m, T - 1:T], ps[:, 1:2])

            # cls columns zero, then add combined cls/modality/pos constants
            x4 = V4(x_sb[:])
            nc.vector.memset(x4[:, :, :, 0:1], 0.0)
            nc.vector.tensor_add(
                x4, x4, xadd_t[:].unsqueeze(2).broadcast_to([128, DC, NB, S]))

            tap("x0", x_sb[:])

            # ---------- helpers ----------
            def rmsnorm(src_ap, tag):
                """feature-major rmsnorm (weight pre-folded): [128,DC,T]"""
                sq = p1.tile([128, DC, T], MMDT, tag="icbuf")
                nc.scalar.activation(sq[:, 0:4, :], src_ap[:, 0:4, :], AF.Square)
                nc.vector.tensor_mul(sq[:, 4:8, :], src_ap[:, 4:8, :],
                                     src_ap[:, 4:8, :])
                psn = psB.tile([128, T], F32, tag="pmisc")
                for k in range(DC):
                    nc.tensor.matmul(psn[:], ones128[:], sq[:, k, :],
                                     start=(k == 0), stop=(k == DC - 1))
                std = p2.tile([128, T], F32, tag="std")
                nc.scalar.activation(std[:], psn[:], AF.Sqrt,
                                     bias=creg(1e-6), scale=creg(1.0 / D))
                rstd = p2.tile([128, T], F32, tag="rstd")
                nc.vector.reciprocal(rstd[:], std[:])
                xn = p2.tile([128, DC, T], MMDT, tag=tag)
                nc.vector.tensor_mul(
                    xn[:], src_ap,
                    rstd[:].unsqueeze(1).broadcast_to([128, DC, T]))
                return xn

            # ---------- layers ----------
            for l in range(L):
                # ---- rmsnorm1 ----
                xn1 = rmsnorm(x_sb[:], "xn")

                if l == 0:
                    tap("xn1", xn1[:])
                # ---- in-proj fused with conv / silu / dt-B-C ----
                xc = p1.tile([128, IC, T], MMDT, tag="icbuf")
                cv = p1.tile([128, IC, T], MMDT, tag="big")
                sg = p1.tile([128, IC, T], MMDT, tag="sm")
                xm = p1.tile([128, IC, T], MMDT, tag="xm")
                gs = p1.tile([128, IC, T], MMDT, tag="gswbc")
                cpk = p2.tile([128, IC, 5], F32, tag="cpack")
                nc.gpsimd.dma_start(out=cpk[:], in_=t_cpack.ap()[l])
                b_in_t = bias_row(t_b_in.ap()[l]) if flags["b_in"] else None
                b_dtbc_t = bias_row(t_b_dtbc.ap()[l]) if flags["b_dtbc"] \
                    else None
                wdt = load_slabs(t_w_dtbc.ap()[l], 4, [128, 4, 192], wdtp,
                                 "wdt")
                psd0 = psA.tile([128, 2, T], F32, tag="pmm", name="psd0")
                psd1 = psA.tile([128, 2, T], F32, tag="pmm", name="psd1")
                cv4a, xc4a = V4(cv[:]), V4(xc[:])
                for mh in range(2):
                    slabs = load_slabs(t_w_in.ap()[l, :, mh], DC, [128, 2048])
                    for mp in range(8):
                        ps = psA.tile([128, 2, T], F32, tag="pmm")
                        for j in range(2):
                            cg = mh * 16 + mp * 2 + j
                            cl = mp * 2 + j
                            for k in range(DC):
                                nc.tensor.matmul(
                                    ps[:, j, :],
                                    slabs[k][:, cl * 128:(cl + 1) * 128],
                                    xn1[:, k, :], start=(k == 0),
                                    stop=(k == DC - 1 and b_in_t is None))
                            if b_in_t is not None:
                                nc.tensor.matmul(
                                    ps[:, j, :],
                                    b_in_t[:, cg * 128:(cg + 1) * 128],
                                    ones_mm[:], start=False, stop=True)
                        c0 = mp * 2
                        if mh == 1:
                            nc.scalar.activation(gs[:, c0:c0 + 2, :], ps[:],
                                                 AF.Sigmoid)
                            continue
                        nc.scalar.copy(xc[:, c0:c0 + 2, :], ps[:])
                        # causal depthwise conv on this chunk pair
                        pr = slice(c0, c0 + 2)
                        xc4 = xc4a[:, pr]
                        cv4 = cv4a[:, pr]

                        def tapw(i, n):
                            return cpk[:, pr, i:i + 1].unsqueeze(3) \
                                .broadcast_to([128, 2, NB, n])

                        nc.vector.tensor_mul(cv4, xc4, tapw(2, S))
                        tsh = p2.tile([128, 2, NB, S - 1], MMDT, tag="tsh")
                        nc.vector.tensor_mul(tsh[:], xc4[:, :, :, 0:S - 1],
                                             tapw(1, S - 1))
                        nc.gpsimd.tensor_add(cv4[:, :, :, 1:S],
                                             cv4[:, :, :, 1:S], tsh[:])
                        tsh2 = p2.tile([128, 2, NB, S - 2], MMDT, tag="tsh2")
                        nc.vector.tensor_mul(tsh2[:], xc4[:, :, :, 0:S - 2],
                                             tapw(0, S - 2))
                        nc.gpsimd.tensor_add(cv4[:, :, :, 2:S],
                                             cv4[:, :, :, 2:S], tsh2[:])
                        if flags["b_conv"]:
                            nc.vector.tensor_add(cv4, cv4, tapw(3, S))
                        # silu
                        nc.scalar.activation(sg[:, pr, :], cv[:, pr, :],
                                             AF.Sigmoid)
                        nc.vector.tensor_mul(xm[:, pr, :], cv[:, pr, :],
                                             sg[:, pr, :])
                        # dt/B/C partial contributions for these k-tiles
                        for kk in (c0, c0 + 1):
                            gi, ii = kk // 4, kk % 4
                            nc.tensor.matmul(
                                psd0[:, 0, :], wdt[gi][:, ii, 0:128],
                                xm[:, kk, :], start=(kk == 0),
                                stop=(kk == 15 and b_dtbc_t is None))
                            nc.tensor.matmul(
                                psd1[0:NS, 0, :], wdt[gi][:, ii, 128:192],
                                xm[:, kk, :], start=(kk == 0),
                                stop=(kk == 15 and b_dtbc_t is None))
                if b_dtbc_t is not None:
                    nc.tensor.matmul(psd0[:, 0, :], b_dtbc_t[:, 0:128],
                                     ones_mm[:], start=False, stop=True)
                    nc.tensor.matmul(psd1[0:NS, 0, :], b_dtbc_t[:, 128:192],
                                     ones_mm[:], start=False, stop=True)

                if l == 0:
                    tap("xm0", xm[:])
                    tap("gs0", gs[:])
                # ---- dt, a = 1-dt, u = dt*B ----
                dt_sb = p2.tile([NS, T], F32, tag="dt")
                nc.scalar.activation(dt_sb[:], psd0[0:NS, 0, :], AF.Sigmoid)
                a_sb = p2.tile([NS, T], F32, tag="a")
                nc.scalar.activation(a_sb[:], psd0[0:NS, 0, :], AF.Sigmoid,
                                     scale=creg(-1.0)[0:NS])
                u_sb = p2.tile([NS, T], F32, tag="u")
                nc.vector.tensor_mul(u_sb[:], psd0[NS:128, 0, :], dt_sb[:])

                # ---- SSM scan (one instruction per batch) ----
                h_sb = p2.tile([NS, T], F32, tag="h")
                for b in range(NB):
                    ts = slice(b * S, (b + 1) * S)
                    nc.vector.tensor_tensor_scan(
                        h_sb[:, ts], a_sb[:, ts], u_sb[:, ts], 0.0,
                        op0=OP.mult, op1=OP.add)
                y_sb = p2.tile([NS, T], F32, tag="y")
                nc.vector.tensor_mul(y_sb[:], h_sb[:], psd1[0:NS, 0, :])
                if l == 0:
                    tap("dt0", dt_sb[:])
                    tap("y0", y_sb[:])

                # ---- layernorm over NS (fused chain) ----
                ysq = p2.tile([NS, T], F32, tag="ysq")
                nc.scalar.activation(ysq[:], y_sb[:], AF.Square)
                psl = psA.tile([128, 2, T], F32, tag="pmm")
                nc.tensor.matmul(psl[:, 0, :], ones64[:], y_sb[:],
                                 start=True, stop=True)
                nc.tensor.matmul(psl[:, 1, :], ones64[:], ysq[:],
                                 start=True, stop=True)
                musq = p2.tile([NS, T], F32, tag="h")
                nc.scalar.activation(musq[:], psl[0:NS, 0, :], AF.Square,
                                     scale=creg(1.0 / NS)[0:NS])
                var = p2.tile([NS, T], F32, tag="ysq2")
                nc.vector.scalar_tensor_tensor(
                    out=var[:], in0=psl[0:NS, 1, :],
                    scalar=creg(1.0 / NS)[0:NS], in1=musq[:],
                    op0=OP.mult, op1=OP.subtract)
                stdl = p2.tile([NS, T], F32, tag="a2")
                nc.scalar.activation(stdl[:], var[:], AF.Sqrt,
                                     bias=creg(1e-5)[0:NS])
                rinv = p2.tile([NS, T], F32, tag="u2")
                nc.vector.reciprocal(rinv[:], stdl[:])
                ytmp = p2.tile([NS, T], F32, tag="dt2")
                nc.vector.scalar_tensor_tensor(
                    out=ytmp[:], in0=psl[0:NS, 0, :],
                    scalar=creg(-1.0 / NS)[0:NS], in1=y_sb[:],
                    op0=OP.mult, op1=OP.add)
                yn = p2.tile([NS + 1, T], MMDT, tag="yn")
                nc.vector.tensor_mul(yn[0:NS, :], ytmp[:], rinv[:])
                nc.vector.memset(yn[NS:NS + 1, :], 1.0)   # bias row for s2i

                if l == 0:
                    tap("yn0", yn[0:NS, :])
                # ---- s2i projection (K=65 incl bias row) ----
                ws2 = wmi.tile([NS + 1, INNER], MMDT, tag="ws2i")
                nc.gpsimd.dma_start(out=ws2[:], in_=t_w_s2i.ap()[l])
                zt = p1.tile([128, IC, T], MMDT, tag="icbuf")
                for mp in range(8):
                    ps = psA.tile([128, 2, T], F32, tag="pmm")
                    for j in range(2):
                        c = mp * 2 + j
                        nc.tensor.matmul(ps[:, j, :],
                                         ws2[:, c * 128:(c + 1) * 128],
                                         yn[:], start=True, stop=True)
                    nc.scalar.copy(zt[:, mp * 2:mp * 2 + 2, :], ps[:])
                    if mp % 4 == 3:
                        # finalize this 8-chunk half: z += D*xm ; z *= gate
                        hh = slice(mp * 2 - 6, mp * 2 + 2)
                        if flags["dp_ones"]:
                            nc.vector.tensor_add(zt[:, hh, :], zt[:, hh, :],
                                                 xm[:, hh, :])
                        else:
                            xmdp = p1.tile([128, IC, T], MMDT, tag="xmdp")
                            nc.vector.tensor_mul(
                                xmdp[:, hh, :], xm[:, hh, :],
                                cpk[:, hh, 4:5].broadcast_to([128, 8, T]))
                            nc.vector.tensor_add(zt[:, hh, :], zt[:, hh, :],
                                                 xmdp[:, hh, :])
                        nc.vector.tensor_mul(zt[:, hh, :], zt[:, hh, :],
                                             gs[:, hh, :])

                if l == 0:
                    tap("zt0", zt[:])
                # ---- out projection + residual ----
                b_out_t = bias_row(t_b_out.ap()[l]) if flags["b_out"] else None
                for mh in range(2):
                    wos = load_slabs(t_w_out.ap()[l, mh], 8, [128, 2, 512],
                                     pool=wsl2, tag="wsl2")
                    for mp in range(2):
                        ps = psA.tile([128, 2, T], F32, tag="pmm")
                        for j in range(2):
                            c = mh * 4 + mp * 2 + j
                            cm = mp * 2 + j
                            for s in range(8):
                                for i in range(2):
                                    nc.tensor.matmul(
                                        ps[:, j, :],
                                        wos[s][:, i, cm * 128:(cm + 1) * 128],
                                        zt[:, s * 2 + i, :],
                                        start=(s == 0 and i == 0),
                                        stop=(s == 7 and i == 1
                                              and b_out_t is None))
                            if b_out_t is not None:
                                nc.tensor.matmul(
                                    ps[:, j, :],
                                    b_out_t[:, c * 128:(c + 1) * 128],
                                    ones_mm[:], start=False, stop=True)
                        c0 = mh * 4 + mp * 2
                        nc.vector.tensor_add(x_sb[:, c0:c0 + 2, :],
                                             x_sb[:, c0:c0 + 2, :], ps[:])

                if l == 0:
                    tap("zt0", zt[:])
                    tap("xmix0", x_sb[:])
                # ---- rmsnorm2 ----
                xn2 = rmsnorm(x_sb[:], "xn")

                # ---- router: exact top-2 softmax via max/sigmoid ----
                wg = wmi.tile([128, DC, E], MMDT, tag="wg")
                nc.gpsimd.dma_start(out=wg[:], in_=t_w_gate.ap()[l])
                bg = bias_row(t_b_gate.ap()[l], tag="bg") if flags["b_gate"] \
                    else None
                lg = p2.tile([S, NB, E], F32, tag="lg")
                for tch in range(NB):
                    ps = psB.tile([128, T], F32, tag="pmisc")
                    psv = ps[0:S, 0:E]
                    for k in range(DC):
                        nc.tensor.matmul(
                            psv, xn2[:, k, tch * S:(tch + 1) * S], wg[:, k, :],
                            start=(k == 0), stop=(k == DC - 1 and bg is None))
                    if bg is not None:
                        nc.tensor.matmul(psv, ones_mm[:, 0:S], bg[:],
                                         start=False, stop=True)
                    nc.scalar.copy(lg[:, tch, :], psv)
                m1 = p2.tile([S, NB, 1], F32, tag="m1")
                nc.vector.tensor_reduce(out=m1[:], in_=lg[:], axis=AX.X,
                                        op=OP.max)
                mask1 = p2.tile([S, NB, E], F32, tag="mask1")
                nc.vector.tensor_tensor(
                    out=mask1[:], in0=lg[:], in1=m1[:].broadcast_to([S, NB, E]),
                    op=OP.is_ge)
                l2t = p2.tile([S, NB, E], F32, tag="l2t")
                nc.vector.tensor_scalar(out=l2t[:], in0=mask1[:], scalar1=-1e9,
                                        scalar2=None, op0=OP.mult)
                nc.vector.tensor_add(l2t[:], l2t[:], lg[:])
                m2 = p2.tile([S, NB, 1], F32, tag="m2")
                nc.vector.tensor_reduce(out=m2[:], in_=l2t[:], axis=AX.X,
                                        op=OP.max)
                dgap = p2.tile([S, NB, 1], F32, tag="dgap")
                nc.vector.tensor_sub(dgap[:], m1[:], m2[:])
                p1t = p2.tile([S, NB, 1], F32, tag="p1t")
                nc.scalar.activation(p1t[:], dgap[:], AF.Sigmoid)
                p2t = p2.tile([S, NB, 1], F32, tag="p2t")
                nc.vector.tensor_scalar(out=p2t[:], in0=p1t[:], scalar1=-1.0,
                                        scalar2=1.0, op0=OP.mult, op1=OP.add)
                mask2 = p2.tile([S, NB, E], F32, tag="mask2")
                nc.vector.tensor_tensor(
                    out=mask2[:], in0=l2t[:], in1=m2[:].broadcast_to([S, NB, E]),
                    op=OP.is_ge)
                wtk = p2.tile([S, NB, E], F32, tag="wtk")
                nc.vector.tensor_mul(wtk[:], mask1[:],
                                     p1t[:].broadcast_to([S, NB, E]))
                wtk2 = p2.tile([S, NB, E], F32, tag="wtk2")
                nc.vector.tensor_mul(wtk2[:], mask2[:],
                                     p2t[:].broadcast_to([S, NB, E]))
                nc.vector.tensor_add(wtk[:], wtk[:], wtk2[:])
                # transpose [98, 8] -> [8, 98], then selector-matmul broadcast
                pst = psB.tile([128, T], F32, tag="pmisc")
                nc.tensor.transpose(
                    pst[0:NB * E, 0:S],
                    wtk[:].rearrange("p b e -> p (b e)"), ident[:])
                wts = p2.tile([NB * E, S], F32, tag="wts")
                nc.scalar.copy(wts[:], pst[0:NB * E, 0:S])
                wbc = p1.tile([128, E, T], F32, tag="gswbc")
                for e in range(E):
                    psw = psB.tile([128, T], F32, tag="pmisc")
                    for b in range(NB):
                        nc.tensor.matmul(
                            psw[:, b * S:(b + 1) * S],
                            sel_t[:, (b * E + e) * 128:(b * E + e + 1) * 128],
                            wts[:], start=True, stop=True)
                    if W8DT != MMDT:
                        nc.scalar.activation(wbc[:, e, :], psw[:], AF.Copy,
                                             scale=creg(1.0 / FSCALE))
                    else:
                        nc.scalar.copy(wbc[:, e, :], psw[:])

                if l == 0:
                    tap("wtk0", wtk[:])
                    tap("wbc0", wbc[:])
                # ---- experts (dense, weighted accumulate) ----
                if W8DT != MMDT:
                    xn2e = p2.tile([128, DC, T], W8DT, tag="xn8")
                    nc.scalar.copy(xn2e[:], xn2[:])
                else:
                    xn2e = xn2
                macc = p1.tile([128, DC, T], F32, tag="sm")
                for e in range(E):
                    b_e1_t = bias_row(t_b_e1.ap()[l, e]) if flags["b_e1"] \
                        else None
                    hg = p1.tile([128, HC, T], W8DT, tag="big")
                    for mh in range(2):
                        slabs = load_slabs(t_w_e1.ap()[l, e, :, mh], DC,
                                           [128, 2048], dt=W8DT)
                        for mp in range(8):
                            ps = psA.tile([128, 2, T], F32, tag="pmm")
                            for j in range(2):
                                cg = mh * 16 + mp * 2 + j
                                cl = mp * 2 + j
                                for k in range(DC):
                                    nc.tensor.matmul(
                                        ps[:, j, :],
                                        slabs[k][:, cl * 128:(cl + 1) * 128],
                                        xn2e[:, k, :], start=(k == 0),
                                        stop=(k == DC - 1 and b_e1_t is None))
                                if b_e1_t is not None:
                                    nc.tensor.matmul(
                                        ps[:, j, :],
                                        b_e1_t[:, cg * 128:(cg + 1) * 128],
                                        ones_mm[:], start=False, stop=True)
                            c0 = mh * 16 + mp * 2
                            if W8DT != MMDT:
                                nc.scalar.activation(hg[:, c0:c0 + 2, :],
                                                     ps[:], AF.Gelu,
                                                     scale=creg(1.0 / FSCALE))
                            else:
                                nc.scalar.activation(hg[:, c0:c0 + 2, :],
                                                     ps[:], AF.Gelu)
                    # w2
                    b_e2_t = bias_row(t_b_e2.ap()[l, e]) if flags["b_e2"] \
                        else None
                    for mh in range(2):
                        w2s = load_slabs(t_w_e2.ap()[l, e, mh], 16,
                                         [128, 2, 512], pool=wsl2, tag="wsl2",
                                         dt=W8DT)
                        for mp in range(2):
                            ps = psA.tile([128, 2, T], F32, tag="pmm")
                            for j in range(2):
                                c = mh * 4 + mp * 2 + j
                                cm = mp * 2 + j
                                for s in range(16):
                                    for i in range(2):
                                        nc.tensor.matmul(
                                            ps[:, j, :],
                                            w2s[s][:, i,
                                                   cm * 128:(cm + 1) * 128],
                                            hg[:, s * 2 + i, :],
                                            start=(s == 0 and i == 0),
                                            stop=(s == 15 and i == 1
                                                  and b_e2_t is None))
                                if b_e2_t is not None:
                                    nc.tensor.matmul(
                                        ps[:, j, :],
                                        b_e2_t[:, c * 128:(c + 1) * 128],
                                        ones_mm[:], start=False, stop=True)
                            c0 = mh * 4 + mp * 2
                            wslice = wbc[:, e:e + 1, :].broadcast_to(
                                [128, 2, T])
                            if e == 0:
                                nc.vector.tensor_mul(
                                    macc[:, c0:c0 + 2, :], ps[:], wslice)
                            else:
                                eot = p2.tile([128, 2, T], F32, tag="eot")
                                nc.vector.tensor_mul(eot[:], ps[:], wslice)
                                nc.vector.tensor_add(
                                    macc[:, c0:c0 + 2, :],
                                    macc[:, c0:c0 + 2, :], eot[:])
                # residual
                nc.vector.tensor_add(x_sb[:], x_sb[:], macc[:])
                tap(f"xlayer{l}", x_sb[:])

            # ---------- head (rmsnorm on cls token + linear) ----------
            xcls = x4[:, :, :, 0:1]                       # [128, 8, 2, 1]
            fsq = p2.tile([128, DC, NB, 1], MMDT, tag="fsq")
            nc.scalar.activation(fsq[:], xcls, AF.Square)
            psf = psB.tile([128, T], F32, tag="pmisc")
            for k in range(DC):
                nc.tensor.matmul(psf[:, 0:NB], ones128[:], fsq[:, k, :, 0],
                                 start=(k == 0), stop=(k == DC - 1))
            fstd = p2.tile([128, NB], F32, tag="fstd")
            nc.scalar.activation(fstd[:], psf[:, 0:NB], AF.Sqrt,
                                 bias=creg(1e-6), scale=creg(1.0 / D))
            frinv = p2.tile([128, NB], F32, tag="frinv")
            nc.vector.reciprocal(frinv[:], fstd[:])
            xf = p2.tile([128, DC, NB, 1], MMDT, tag="xf")
            nc.vector.tensor_mul(
                xf[:], xcls,
                frinv[:].unsqueeze(1).unsqueeze(3).broadcast_to(
                    [128, DC, NB, 1]))
            bhd = bias_row(t_b_hd.ap(), tag="bg") if flags["b_hd"] else None
            psh = psB.tile([128, T], F32, tag="pmisc")
            for k in range(DC):
                nc.tensor.matmul(psh[0:NCLS, 0:NB], whd[:, k, :], xf[:, k, :, 0],
                                 start=(k == 0),
                                 stop=(k == DC - 1 and bhd is None))
            if bhd is not None:
                nc.tensor.matmul(psh[0:NCLS, 0:NB], bhd[:], ones_mm[:, 0:NB],
                                 start=False, stop=True)
            out_sb = p2.tile([NCLS, NB], F32, tag="osb")
            nc.scalar.copy(out_sb[:], psh[0:NCLS, 0:NB])
            nc.sync.dma_start(out=t_out.ap(), in_=out_sb[:])

    nc.compile()
    return nc


def get_nc(flags):
    key = (os.environ.get("BASS_MM_DT", "bf16"), tuple(sorted(flags.items())))
    if key not in _CACHE:
        _CACHE[key] = _build(tuple(sorted(flags.items())))
    return _CACHE[key]


def kernel(**inputs):
    from concourse.bass_utils import run_bass_kernel_spmd
    sh, percore, flags = _prep(inputs)
    nc = get_nc(flags)
    in_maps = [{**sh, **pc} for pc in percore]
    res = run_bass_kernel_spmd(nc, in_maps, core_ids=list(range(NCORES)))
    outs = [res.results[c]["out"].T for c in range(NCORES)]   # [2, 13] each
    return np.ascontiguousarray(
        np.concatenate(outs, axis=0)).astype(np.float32)



# revision 18
# speedup vs baseline: 3.1994x; 3.1994x over previous
"""Trainium2 Bass kernel for nn_MixtureOfMambaModel.

Exact graph-level optimization: the classifier head reads x[:, 0] (the cls
token), and every sequence-mixing op in the model is causal (depthwise conv
with left-only padding, forward SSM scan) or per-token (norms, MoE, router).
Token 0 therefore never observes tokens 1..97, and its initial value is
cls_token + modality_embed[:,3] + pos_embed[:,0] — independent of the video /
audio / question inputs. The model output is a function of the weights only,
identical across the batch. The kernel computes that single-token forward
pass exactly, on device, and broadcasts the result to all 16 batch rows.

Device strategy (8 NeuronCores, tensor-parallel single-token forward):
  - All big projections are split 8 ways: in_proj / expert-w1 by output
    columns, out_proj / expert-w2 by contraction rows. The [1024] activation
    vector is replicated as a [128, 8] tile on every core.
  - Three 4KB AllReduces per layer stitch the partials together:
    dt/B/C projections [192], mixer output [1024], weighted MoE output
    [1024]. Collectives run on internal DRAM tiles (CCE fp32 add).
  - Small/serial pieces (rmsnorm, SSM step at t=0, layernorm over 64,
    router top-2, conv tap) are replicated on every core — they are a few
    hundred elements each.
  - Matmuls run stationary-weight with a 1-column moving operand (the
    token), bf16 in / fp32 PSUM accumulation. Biases and norm weights are
    folded host-side exactly as in the dense formulation.
"""

import numpy as np
import ml_dtypes

# ---- model dims (hardcoded per spec) ----
B = 16
D = 1024
INNER = 2048
NS = 64
HID = 4096
E = 4
L = 4
NCLS = 13
DC = D // 128                # 8 chunks of the model dim
NCORES = 8
CIN = 2 * INNER // NCORES // 128   # in-proj col chunks per core (4)
CXM = INNER // NCORES // 128       # xm col chunks per core (2)
CH = HID // NCORES // 128          # expert hidden chunks per core (4)

BF16 = ml_dtypes.bfloat16

_CACHE = {}


# --------------------------------------------------------------------------
# Host-side preparation: slicing / layout / constant folding on weights.
# --------------------------------------------------------------------------

def _prep(inputs):
    f32 = np.float32
    g = {k: np.asarray(v) for k, v in inputs.items()}

    # token-0 initial value: cls + modality_embed[3] + pos_embed[0]
    x0 = (np.asarray(g["cls_token"][0, 0], f32)
          + np.asarray(g["modality_embed"][0, 3], f32)
          + np.asarray(g["pos_embed"][0, 0], f32))            # [1024]

    sh = {}
    sh["x0"] = np.ascontiguousarray(x0.reshape(DC, 128).T).astype(f32)  # [128, 8]

    w_in = (g["in_w"] * g["norm1_w"][:, :, None]).astype(f32)  # [L,1024,4096]
    w_gate = (g["gate_w"] * g["norm2_w"][:, :, None]).astype(f32)
    w_e1 = (g["e_w1"] * g["norm2_w"][:, None, :, None]).astype(f32)
    w_hd = (g["head_w"] * g["fnorm_w"][:, None]).astype(f32)   # [1024, 13]

    # replicated (shared) tensors
    sh["w_gate"] = np.ascontiguousarray(
        w_gate.reshape(L, DC, 128, E).transpose(0, 2, 1, 3)).astype(BF16)
    sh["b_gate"] = g["gate_b"].reshape(L, 1, E).astype(f32)
    sh["b_dtbc"] = np.ascontiguousarray(
        np.stack([g["dt_b"], g["Bp_b"], g["Cp_b"]], axis=2)).astype(f32)
    sh["b_out"] = np.ascontiguousarray(
        g["out_b"].reshape(L, DC, 128).transpose(0, 2, 1)).astype(f32)
    sh["b_e2"] = np.ascontiguousarray(
        g["e_b2"].reshape(L, E, DC, 128).transpose(0, 3, 1, 2)).astype(f32)
    sh["w_hd"] = np.ascontiguousarray(
        w_hd.reshape(DC, 128, NCLS).transpose(1, 0, 2)).astype(BF16)
    sh["b_hd"] = g["head_b"].reshape(1, NCLS).astype(f32)

    percore = []
    for c in range(NCORES):
        pc = {}
        mcols = slice(c * 256, (c + 1) * 256)                  # xm cols
        gcols = slice(INNER + c * 256, INNER + (c + 1) * 256)  # gate cols
        hcols = slice(c * 512, (c + 1) * 512)                  # hidden cols

        wi = np.concatenate([w_in[:, :, mcols], w_in[:, :, gcols]], axis=2)
        # [L, 1024, 512] -> [L, 128p, 8k, 4j, 128m]
        pc["w_in"] = np.ascontiguousarray(
            wi.reshape(L, DC, 128, CIN, 128).transpose(0, 2, 1, 3, 4)
        ).astype(BF16)
        bi = np.concatenate([g["in_b"][:, mcols], g["in_b"][:, gcols]], 1)
        pc["b_in"] = np.ascontiguousarray(
            bi.reshape(L, CIN, 128).transpose(0, 2, 1)).astype(f32)

        cpk = np.zeros((L, 128, CXM, 3), f32)
        cpk[:, :, :, 0] = g["conv_w"][:, mcols, 0, 2].reshape(
            L, CXM, 128).transpose(0, 2, 1)
        cpk[:, :, :, 1] = g["conv_b"][:, mcols].reshape(
            L, CXM, 128).transpose(0, 2, 1)
        cpk[:, :, :, 2] = g["D_param"][:, mcols].reshape(
            L, CXM, 128).transpose(0, 2, 1)
        pc["cpk"] = cpk

        wd = np.concatenate([g["dt_w"], g["Bp_w"], g["Cp_w"]], 2)[:, mcols]
        pc["w_dtbc"] = np.ascontiguousarray(
            wd.reshape(L, CXM, 128, 3 * NS).transpose(0, 2, 1, 3)
        ).astype(BF16)                                         # [L,128,2,192]

        s2 = np.concatenate(
            [g["s2i_w"][:, :, mcols], g["s2i_b"][:, None, mcols]], 1)
        pc["w_s2i"] = np.ascontiguousarray(s2).astype(BF16)    # [L, 65, 256]

        pc["w_out"] = np.ascontiguousarray(
            g["out_w"][:, mcols].reshape(L, CXM, 128, DC, 128)
            .transpose(0, 2, 1, 3, 4)).astype(BF16)            # [L,128,2,8,128]

        pc["w_e1"] = np.ascontiguousarray(
            w_e1[:, :, :, hcols].reshape(L, E, DC, 128, CH, 128)
            .transpose(0, 1, 3, 2, 4, 5)).astype(BF16)         # [L,E,128,8,4,128]
        pc["b_e1"] = np.ascontiguousarray(
            g["e_b1"][:, :, hcols].reshape(L, E, CH, 128)
            .transpose(0, 1, 3, 2)).astype(f32)                # [L,E,128,4]
        pc["w_e2"] = np.ascontiguousarray(
            g["e_w2"][:, :, hcols].reshape(L, E, CH, 128, DC, 128)
            .transpose(0, 1, 3, 2, 4, 5)).astype(BF16)         # [L,E,128,4,8,128]
        percore.append(pc)

    flags = {}
    return sh, percore, flags


# --------------------------------------------------------------------------
# Device kernel builder
# --------------------------------------------------------------------------

def _build():
    import concourse.mybir as mybir
    import concourse.tile as tile
    from concourse import bacc

    F32 = mybir.dt.float32
    BF = mybir.dt.bfloat16
    AF = mybir.ActivationFunctionType
    OP = mybir.AluOpType
    AX = mybir.AxisListType
    RG = [list(range(NCORES))]

    nc = bacc.Bacc("TRN2", target_bir_lowering=False, debug=False,
                   num_devices=NCORES)

    def din(name, shape, dt=BF):
        return nc.dram_tensor(name, list(shape), dt, kind="ExternalInput")

    t_x0 = din("x0", [128, DC], F32)
    t_w_in = din("w_in", [L, 128, DC, CIN, 128])
    t_b_in = din("b_in", [L, 128, CIN], F32)
    t_cpk = din("cpk", [L, 128, CXM, 3], F32)
    t_w_dtbc = din("w_dtbc", [L, 128, CXM, 3 * NS])
    t_b_dtbc = din("b_dtbc", [L, NS, 3], F32)
    t_w_s2i = din("w_s2i", [L, NS + 1, 256])
    t_w_out = din("w_out", [L, 128, CXM, DC, 128])
    t_b_out = din("b_out", [L, 128, DC], F32)
    t_w_gate = din("w_gate", [L, 128, DC, E])
    t_b_gate = din("b_gate", [L, 1, E], F32)
    t_w_e1 = din("w_e1", [L, E, 128, DC, CH, 128])
    t_b_e1 = din("b_e1", [L, E, 128, CH], F32)
    t_w_e2 = din("w_e2", [L, E, 128, CH, DC, 128])
    t_b_e2 = din("b_e2", [L, 128, E, DC], F32)
    t_w_hd = din("w_hd", [128, DC, NCLS])
    t_b_hd = din("b_hd", [1, NCLS], F32)
    t_out = nc.dram_tensor("out", [1, NCLS], F32, kind="ExternalOutput")

    with tile.TileContext(nc) as tc:
        with tc.tile_pool(name="consts", bufs=1) as consts, \
             tc.tile_pool(name="wi", bufs=2) as wip, \
             tc.tile_pool(name="wsm", bufs=2) as wsm, \
             tc.tile_pool(name="wo", bufs=2) as wop, \
             tc.tile_pool(name="we1", bufs=5) as we1p, \
             tc.tile_pool(name="we2", bufs=5) as we2p, \
             tc.tile_pool(name="bia", bufs=2) as biap, \
             tc.tile_pool(name="act", bufs=2) as actp, \
             tc.tile_pool(name="ps", bufs=1, space="PSUM") as psp, \
             tc.tile_pool(name="ard", bufs=4, space="DRAM") as ardp:

            ones_p = consts.tile([128, 1], BF)      # partition-sum lhsT
            nc.vector.memset(ones_p[:], 1.0)
            ones_pf = consts.tile([128, 1], F32)    # f32 partition-sum lhsT
            nc.vector.memset(ones_pf[:], 1.0)
            ones_b = consts.tile([1, 128], F32)     # broadcast lhsT (K=1)
            nc.vector.memset(ones_b[:], 1.0)

            _cregs = {}

            def creg(val, p=128):
                key = (val, p)
                if key not in _cregs:
                    ct = consts.tile([p, 1], F32, tag=f"c{len(_cregs)}")
                    nc.vector.memset(ct[:], val)
                    _cregs[key] = ct
                return _cregs[key][:]

            x_sb = consts.tile([128, DC], F32, tag="x")
            nc.sync.dma_start(out=x_sb[:], in_=t_x0.ap())

            def rmsnorm(src, tag):
                """replicated rmsnorm of the [128, 8] vector -> bf16"""
                sq = actp.tile([128, DC], BF, tag=tag + "sq")
                nc.vector.tensor_mul(sq[:], src, src)
                pssum = psp.tile([128, DC], F32, tag="pmini")
                nc.tensor.matmul(pssum[0:1, :], ones_p[:], sq[:],
                                 start=True, stop=True)
                rs = actp.tile([1, 1], F32, tag=tag + "rs")
                nc.vector.tensor_reduce(out=rs[:], in_=pssum[0:1, :],
                                        axis=AX.X, op=OP.add)
                psb = psp.tile([128, DC], F32, tag="pmini")
                nc.tensor.matmul(psb[:, 0:1], ones_b[:], rs[:],
                                 start=True, stop=True)
                std = actp.tile([128, 1], F32, tag=tag + "std")
                nc.scalar.activation(std[:], psb[:, 0:1], AF.Sqrt,
                                     bias=creg(1e-6), scale=creg(1.0 / D))
                rinv = actp.tile([128, 1], F32, tag=tag + "ri")
                nc.vector.reciprocal(rinv[:], std[:])
                xn = actp.tile([128, DC], BF, tag=tag)
                nc.vector.tensor_mul(xn[:], src,
                                     rinv[:].broadcast_to([128, DC]))
                return xn

            for l in range(L):
                # ---------- mixer ----------
                xn1 = rmsnorm(x_sb[:], "xn1")

                wi = wip.tile([128, DC, CIN, 128], BF, tag="wi")
                nc.sync.dma_start(out=wi[:], in_=t_w_in.ap()[l])
                bi = biap.tile([128, CIN], F32, tag="bi")
                nc.sync.dma_start(out=bi[:], in_=t_b_in.ap()[l])
                cpk = biap.tile([128, CXM, 3], F32, tag="cpk")
                nc.sync.dma_start(out=cpk[:], in_=t_cpk.ap()[l])

                pin = psp.tile([128, CIN], F32, tag="pin")
                for j in range(CIN):
                    for k in range(DC):
                        nc.tensor.matmul(pin[:, j:j + 1], wi[:, k, j, :],
                                         xn1[:, k:k + 1], start=(k == 0),
                                         stop=(k == DC - 1))

                # conv tap at t=0 + silu on xm cols; sigmoid on gate cols
                xmp = actp.tile([128, CXM], F32, tag="xmp")
                nc.vector.tensor_add(xmp[:], pin[:, 0:CXM], bi[:, 0:CXM])
                nc.vector.tensor_mul(xmp[:], xmp[:], cpk[:, :, 0])
                nc.vector.tensor_add(xmp[:], xmp[:], cpk[:, :, 1])
                sgm = actp.tile([128, CXM], F32, tag="sgm")
                nc.scalar.activation(sgm[:], xmp[:], AF.Sigmoid)
                xm = actp.tile([128, CXM], F32, tag="xm")
                nc.vector.tensor_mul(xm[:], xmp[:], sgm[:])
                xmb = actp.tile([128, CXM], BF, tag="xmb")
                nc.scalar.copy(xmb[:], xm[:])
                gt = actp.tile([128, CXM], F32, tag="gt")
                nc.vector.tensor_add(gt[:], pin[:, CXM:CIN], bi[:, CXM:CIN])
                gsig = actp.tile([128, CXM], F32, tag="gsig")
                nc.scalar.activation(gsig[:], gt[:], AF.Sigmoid)

                # dt/B/C partial projections over this core's xm slice
                wd = wsm.tile([128, CXM, 3 * NS], BF, tag="wd")
                nc.sync.dma_start(out=wd[:], in_=t_w_dtbc.ap()[l])
                pd = psp.tile([128, 2], F32, tag="pd")
                for k in range(CXM):
                    nc.tensor.matmul(pd[:, 0:1], wd[:, k, 0:128],
                                     xmb[:, k:k + 1], start=(k == 0),
                                     stop=(k == CXM - 1))
                    nc.tensor.matmul(pd[0:NS, 1:2], wd[:, k, 128:192],
                                     xmb[:, k:k + 1], start=(k == 0),
                                     stop=(k == CXM - 1))

                ar1s = actp.tile([128, 2], F32, tag="ar1s")
                nc.vector.memset(ar1s[:], 0.0)
                nc.scalar.copy(ar1s[:, 0:1], pd[:, 0:1])
                nc.scalar.copy(ar1s[0:NS, 1:2], pd[0:NS, 1:2])
                ar1 = ardp.tile([128, 2], F32, tag="ar1")
                nc.sync.dma_start(out=ar1[:], in_=ar1s[:])
                nc.gpsimd.collective_compute(
                    "AllReduce", OP.add, replica_groups=RG,
                    ins=[ar1[:]], outs=[ar1[:]])
                dtbc = actp.tile([NS, 3], F32, tag="dtbc")
                nc.sync.dma_start(out=dtbc[:, 0:1], in_=ar1[0:NS, 0:1])
                nc.sync.dma_start(out=dtbc[:, 1:2], in_=ar1[NS:128, 0:1])
                nc.sync.dma_start(out=dtbc[:, 2:3], in_=ar1[0:NS, 1:2])
                bdt = biap.tile([NS, 3], F32, tag="bdt")
                nc.sync.dma_start(out=bdt[:], in_=t_b_dtbc.ap()[l])
                nc.vector.tensor_add(dtbc[:], dtbc[:], bdt[:])

                # SSM at t=0: state = dt*B ; y = C*state ; LN over 64
                dt_t = actp.tile([NS, 1], F32, tag="dt")
                nc.scalar.activation(dt_t[:], dtbc[:, 0:1], AF.Sigmoid)
                y_t = actp.tile([NS, 2], F32, tag="y")
                nc.vector.tensor_mul(y_t[:, 0:1], dt_t[:], dtbc[:, 1:2])
                nc.vector.tensor_mul(y_t[:, 0:1], y_t[:, 0:1], dtbc[:, 2:3])
                nc.vector.tensor_mul(y_t[:, 1:2], y_t[:, 0:1], y_t[:, 0:1])
                psl = psp.tile([128, 2], F32, tag="pmini2")
                nc.tensor.matmul(psl[0:1, :], ones_pf[0:NS, :], y_t[:],
                                 start=True, stop=True)
                mu = actp.tile([1, 2], F32, tag="mu")   # [mean, mean-of-sq]
                nc.vector.tensor_scalar(out=mu[:], in0=psl[0:1, :],
                                        scalar1=1.0 / NS, scalar2=None,
                                        op0=OP.mult)
                var = actp.tile([1, 1], F32, tag="var")
                nc.vector.tensor_mul(var[:], mu[:, 0:1], mu[:, 0:1])
                nc.vector.tensor_sub(var[:], mu[:, 1:2], var[:])
                stdl = actp.tile([1, 1], F32, tag="stdl")
                nc.scalar.activation(stdl[:], var[:], AF.Sqrt,
                                     bias=creg(1e-5, 1))
                ri = actp.tile([1, 2], F32, tag="ri2")  # [rstd, mean]
                nc.vector.reciprocal(ri[:, 0:1], stdl[:])
                nc.scalar.copy(ri[:, 1:2], mu[:, 0:1])
                psb2 = psp.tile([128, 2], F32, tag="pmini2")
                nc.tensor.matmul(psb2[0:NS, :], ones_b[:, 0:NS], ri[:],
                                 start=True, stop=True)
                yn = actp.tile([NS + 1, 1], BF, tag="yn")
                ytmp = actp.tile([NS, 1], F32, tag="ytmp")
                nc.vector.tensor_sub(ytmp[:], y_t[:, 0:1], psb2[0:NS, 1:2])
                nc.vector.tensor_mul(ytmp[:], ytmp[:], psb2[0:NS, 0:1])
                nc.scalar.copy(yn[0:NS, :], ytmp[:])
                nc.vector.memset(yn[NS:NS + 1, :], 1.0)

                # s2i (+bias row) + D*xm, gated; then out-proj partial
                ws2 = wsm.tile([NS + 1, 256], BF, tag="ws2")
                nc.sync.dma_start(out=ws2[:], in_=t_w_s2i.ap()[l])
                pz = psp.tile([128, CXM], F32, tag="pd")
                for j in range(CXM):
                    nc.tensor.matmul(pz[:, j:j + 1],
                                     ws2[:, j * 128:(j + 1) * 128],
                                     yn[:], start=True, stop=True)
                z = actp.tile([128, CXM], F32, tag="z")
                nc.vector.tensor_mul(z[:], xm[:], cpk[:, :, 2])
                nc.vector.tensor_add(z[:], z[:], pz[:])
                nc.vector.tensor_mul(z[:], z[:], gsig[:])
                zb = actp.tile([128, CXM], BF, tag="zb")
                nc.scalar.copy(zb[:], z[:])

                wo = wop.tile([128, CXM, DC, 128], BF, tag="wo")
                nc.sync.dma_start(out=wo[:], in_=t_w_out.ap()[l])
                po = psp.tile([128, DC], F32, tag="po")
                for m in range(DC):
                    for k in range(CXM):
                        nc.tensor.matmul(po[:, m:m + 1], wo[:, k, m, :],
                                         zb[:, k:k + 1], start=(k == 0),
                                         stop=(k == CXM - 1))
                ar2s = actp.tile([128, DC], F32, tag="ar2s")
                nc.scalar.copy(ar2s[:], po[:])
                ar2 = ardp.tile([128, DC], F32, tag="ar2")
                nc.sync.dma_start(out=ar2[:], in_=ar2s[:])
                nc.gpsimd.collective_compute(
                    "AllReduce", OP.add, replica_groups=RG,
                    ins=[ar2[:]], outs=[ar2[:]])
                mix = actp.tile([128, DC], F32, tag="mix")
                nc.sync.dma_start(out=mix[:], in_=ar2[:])
                bo = biap.tile([128, DC], F32, tag="bo")
                nc.sync.dma_start(out=bo[:], in_=t_b_out.ap()[l])
                nc.vector.tensor_add(mix[:], mix[:], bo[:])
                nc.vector.tensor_add(x_sb[:], x_sb[:], mix[:])

                # ---------- MoE ----------
                xn2 = rmsnorm(x_sb[:], "xn2")

                wg = wsm.tile([128, DC, E], BF, tag="wg")
                nc.sync.dma_start(out=wg[:], in_=t_w_gate.ap()[l])
                bg = biap.tile([1, E], F32, tag="bg")
                nc.sync.dma_start(out=bg[:], in_=t_b_gate.ap()[l])
                pg = psp.tile([128, E], F32, tag="pmini")
                for k in range(DC):
                    nc.tensor.matmul(pg[0:1, :], xn2[:, k:k + 1], wg[:, k, :],
                                     start=(k == 0), stop=(k == DC - 1))
                lg = actp.tile([1, E], F32, tag="lg")
                nc.vector.tensor_add(lg[:], pg[0:1, :], bg[:])
                m1 = actp.tile([1, 1], F32, tag="m1")
                nc.vector.tensor_reduce(out=m1[:], in_=lg[:], axis=AX.X,
                                        op=OP.max)
                mask1 = actp.tile([1, E], F32, tag="mask1")
                nc.vector.tensor_tensor(out=mask1[:], in0=lg[:],
                                        in1=m1[:].broadcast_to([1, E]),
                                        op=OP.is_ge)
                l2 = actp.tile([1, E], F32, tag="l2")
                nc.vector.scalar_tensor_tensor(
                    out=l2[:], in0=mask1[:], scalar=creg(-1e9, 1), in1=lg[:],
                    op0=OP.mult, op1=OP.add)
                m2 = actp.tile([1, 1], F32, tag="m2")
                nc.vector.tensor_reduce(out=m2[:], in_=l2[:], axis=AX.X,
                                        op=OP.max)
                dgap = actp.tile([1, 1], F32, tag="dgap")
                nc.vector.tensor_sub(dgap[:], m1[:], m2[:])
                p1 = actp.tile([1, 1], F32, tag="p1")
                nc.scalar.activation(p1[:], dgap[:], AF.Sigmoid)
                p2 = actp.tile([1, 1], F32, tag="p2")
                nc.vector.tensor_scalar(out=p2[:], in0=p1[:], scalar1=-1.0,
                                        scalar2=1.0, op0=OP.mult, op1=OP.add)
                mask2 = actp.tile([1, E], F32, tag="mask2")
                nc.vector.tensor_tensor(out=mask2[:], in0=l2[:],
                                        in1=m2[:].broadcast_to([1, E]),
                                        op=OP.is_ge)
                wsel = actp.tile([1, E], F32, tag="wsel")
                nc.vector.tensor_mul(wsel[:], mask1[:],
                                     p1[:].broadcast_to([1, E]))
                wsel2 = actp.tile([1, E], F32, tag="wsel2")
                nc.vector.tensor_mul(wsel2[:], mask2[:],
                                     p2[:].broadcast_to([1, E]))
                nc.vector.tensor_add(wsel[:], wsel[:], wsel2[:])
                pgb = psp.tile([128, E], F32, tag="pmini")
                nc.tensor.matmul(pgb[:], ones_b[:], wsel[:],
                                 start=True, stop=True)
                wbc = actp.tile([128, E], F32, tag="wbc")
                nc.scalar.copy(wbc[:], pgb[:])

                macc = actp.tile([128, DC], F32, tag="macc")
                b2w = actp.tile([128, DC], F32, tag="b2w")
                be2 = biap.tile([128, E, DC], F32, tag="be2")
                nc.sync.dma_start(out=be2[:], in_=t_b_e2.ap()[l])
                for e in range(E):
                    w1 = we1p.tile([128, DC, CH, 128], BF, tag="we1")
                    nc.sync.dma_start(out=w1[:], in_=t_w_e1.ap()[l, e])
                    be1 = biap.tile([128, CH], F32, tag="be1")
                    nc.sync.dma_start(out=be1[:], in_=t_b_e1.ap()[l, e])
                    ph = psp.tile([128, CH], F32, tag="ph", bufs=1)
                    for j in range(CH):
                        for k in range(DC):
                            nc.tensor.matmul(ph[:, j:j + 1], w1[:, k, j, :],
                                             xn2[:, k:k + 1], start=(k == 0),
                                             stop=(k == DC - 1))
                    hsum = actp.tile([128, CH], F32, tag="hsum")
                    nc.vector.tensor_add(hsum[:], ph[:], be1[:])
                    hg = actp.tile([128, CH], BF, tag="hg")
                    nc.scalar.activation(hg[:], hsum[:], AF.Gelu)

                    w2 = we2p.tile([128, CH, DC, 128], BF, tag="we2")
                    nc.sync.dma_start(out=w2[:], in_=t_w_e2.ap()[l, e])
                    pe2 = psp.tile([128, DC], F32, tag="pe2", bufs=2)
                    for m in range(DC):
                        for k in range(CH):
                            nc.tensor.matmul(pe2[:, m:m + 1], w2[:, k, m, :],
                                             hg[:, k:k + 1], start=(k == 0),
                                             stop=(k == CH - 1))
                    if e == 0:
                        nc.vector.scalar_tensor_tensor(
                            out=macc[:], in0=pe2[:], scalar=wbc[:, 0:1],
                            in1=x_sb[:], op0=OP.mult, op1=OP.bypass)
                        nc.vector.scalar_tensor_tensor(
                            out=b2w[:], in0=be2[:, 0, :], scalar=wbc[:, 0:1],
                            in1=be2[:, 0, :], op0=OP.mult, op1=OP.bypass)
                    else:
                        nc.vector.scalar_tensor_tensor(
                            out=macc[:], in0=pe2[:], scalar=wbc[:, e:e + 1],
                            in1=macc[:], op0=OP.mult, op1=OP.add)
                        nc.vector.scalar_tensor_tensor(
                            out=b2w[:], in0=be2[:, e, :],
                            scalar=wbc[:, e:e + 1],
                            in1=b2w[:], op0=OP.mult, op1=OP.add)

                ar3 = ardp.tile([128, DC], F32, tag="ar3")
                nc.sync.dma_start(out=ar3[:], in_=macc[:])
                nc.gpsimd.collective_compute(
                    "AllReduce", OP.add, replica_groups=RG,
                    ins=[ar3[:]], outs=[ar3[:]])
                moe = actp.tile([128, DC], F32, tag="moe")
                nc.sync.dma_start(out=moe[:], in_=ar3[:])
                nc.vector.tensor_add(moe[:], moe[:], b2w[:])
                nc.vector.tensor_add(x_sb[:], x_sb[:], moe[:])

            # ---------- head ----------
            xf = rmsnorm(x_sb[:], "xf")
            whd = consts.tile([128, DC, NCLS], BF, tag="whd")
            nc.sync.dma_start(out=whd[:], in_=t_w_hd.ap())
            bhd = consts.tile([1, NCLS], F32, tag="bhd")
            nc.sync.dma_start(out=bhd[:], in_=t_b_hd.ap())
            phd = psp.tile([128, NCLS], F32, tag="pmini")
            for k in range(DC):
                nc.tensor.matmul(phd[0:1, :], xf[:, k:k + 1], whd[:, k, :],
                                 start=(k == 0), stop=(k == DC - 1))
            osb = actp.tile([1, NCLS], F32, tag="osb")
            nc.vector.tensor_add(osb[:], phd[0:1, :], bhd[:])
            nc.sync.dma_start(out=t_out.ap(), in_=osb[:])

    nc.compile()
    return nc


def get_nc(flags):
    if "nc" not in _CACHE:
        _CACHE["nc"] = _build()
    return _CACHE["nc"]


def kernel(**inputs):
    from concourse.bass_utils import run_bass_kernel_spmd
    sh, percore, flags = _prep(inputs)
    nc = get_nc(flags)
    in_maps = [{**sh, **pc} for pc in percore]
    res = run_bass_kernel_spmd(nc, in_maps, core_ids=list(range(NCORES)))
    row = np.asarray(res.results[0]["out"], np.float32).reshape(NCLS)
    return np.ascontiguousarray(
        np.broadcast_to(row[None, :], (B, NCLS))).astype(np.float32)
